# revision 1
# baseline (speedup 1.0000x reference)
"""Trainium2 Bass kernel for CosyVoice3 DiT attention (B=2, S=2048, H=16, hd=64, D=1024).

Sharding: tensor parallelism over heads - 2 heads per core on 8 cores.
Each core computes QKV projections for its head slice, RoPE, full attention
for its 2 heads, then its heads' contribution to the output projection
(row-parallel). The host gather sums the 8 partial outputs and adds biases.

Key layout/scheduling choices (v2):
  - x is transposed on the HOST to xT [D, T]; no on-chip transposes of x.
  - RoPE pair-swap via DVE stream_shuffle (32-lane partition permute), not a
    PE matmul; sin is sign-folded host-side, so q_rot = q*cos + shuf(q)*sin'.
  - The V projection bias commutes through softmax attention (out = attn+bv),
    so it is folded into the output bias on the host: bo' = bo + bv @ Wo.
  - Everything runs transposed ([dim, tokens]); scoresT = K @ Q^T per k-tile,
    both heads packed into disjoint PE row groups via tile_position.
    V_aug = [V | ones] gives the softmax denominator in output row 64.
  - Emission interleaves: (b0,qc0) attention starts right after chunk 0's
    projections; later chunks' projections and o-proj matmuls are pumped as
    fillers inside the exp-paced attention slot loop so the PE, ACT, and DVE
    queues all stay busy. exp on ACT is the pacing engine (~1us per k-tile).
  - Partial outputs leave as bf16; the host sums the 8 partials in fp32.
"""
import sys
sys.path.insert(0, "/opt/trn_rl_repo")
from collections import deque
import numpy as np

# NTFF profile hook shim: this image's antenv lacks axon_hooks, which
# bass_utils imports unconditionally when trace=True (and the boot-time
# installer degrades silently without it). Recreate the module and install
# the ctypes-based hook so neuron-profile traces work.
import types as _types
try:
    import antenv as _antenv
    if "antenv.axon_hooks" not in sys.modules:
        _hooks = _types.ModuleType("antenv.axon_hooks")
        _hook_box = [None]
        _hooks.set_axon_ntff_profile_hook = lambda h: _hook_box.__setitem__(0, h)
        _hooks.get_axon_ntff_profile_hook = lambda: _hook_box[0]
        sys.modules["antenv.axon_hooks"] = _hooks
        _antenv.axon_hooks = _hooks
        try:
            from trn_agent_boot.trn_boot import _ntff_profile_via_ctypes
            _hooks.set_axon_ntff_profile_hook(
                _ntff_profile_via_ctypes("/opt/axon/libaxon_pjrt.so"))
        except Exception:
            pass
except Exception:
    pass

import concourse.bass as bass
import concourse.mybir as mybir
from concourse import bacc
from concourse.tile import TileContext
from concourse.bass_interp import get_hw_module
from concourse import bass_utils
from concourse.masks import make_identity
bass_utils.upload_artifacts = lambda tmpdir: str(tmpdir)  # no S3 in container

# constants (hardcoded per problem spec)
B, S, D, H, HD = 2, 2048, 1024, 16, 64
T = B * S                 # 4096 tokens
NC = 8                    # cores
HPC = H // NC             # 2 heads per core
CW = HPC * HD             # 128 rows/cols per core
SCALE = 1.0 / np.sqrt(HD)
F32 = mybir.dt.float32
F32R = mybir.dt.float32r
BF16 = mybir.dt.bfloat16
AF = mybir.ActivationFunctionType

NCHUNK = 8                # token chunks of 512
CH = T // NCHUNK          # 512
QW = 512                  # q chunk width
QC = S // QW              # 4 q chunks per batch
KT = S // 128             # 16 k-tiles per batch
DC = D // 128             # 8 contraction tiles

_CACHE = {}


def _build(use_mask: bool):
    nc = bacc.Bacc("TRN2", target_bir_lowering=False, debug=False, num_devices=NC)

    # x and the QKV weights stream as bf16: halves the dominant DMA traffic
    # (the hwdge queues sustain only ~100-170 GB/s each) and bf16 LDWEIGHTS
    # run 3x faster than fp32r ones
    xT_d = nc.dram_tensor("xt", [D, T], BF16, kind="ExternalInput")
    # host pre-arranges projection weights to [128, DC*CW] so the load is a
    # plain contiguous-row DMA (512B-elem rearranged loads cost ~5us of
    # descriptor generation on the sync queue)
    wq_d = nc.dram_tensor("wq", [128, DC * CW], BF16, kind="ExternalInput")
    wk_d = nc.dram_tensor("wk", [128, DC * CW], BF16, kind="ExternalInput")
    wv_d = nc.dram_tensor("wv", [128, DC * CW], BF16, kind="ExternalInput")
    wo_d = nc.dram_tensor("wo", [CW, D], BF16, kind="ExternalInput")
    bq_d = nc.dram_tensor("bq", [CW, 1], F32, kind="ExternalInput")
    bk_d = nc.dram_tensor("bk", [CW, 1], F32, kind="ExternalInput")
    # cos and sign-folded sin packed side by side: one DMA per chunk
    cs_d = nc.dram_tensor("cst", [CW, 2 * T], BF16, kind="ExternalInput")
    if use_mask:
        mt_d = nc.dram_tensor("maskt", [S, S], F32, kind="ExternalInput")

    ypT_d = nc.dram_tensor("ypT", [D, T], BF16, kind="ExternalOutput")

    SWAP_MASK = [i ^ 1 for i in range(32)]

    with TileContext(nc) as tc:
        with tc.tile_pool(name="persist", bufs=1) as persist, \
             tc.tile_pool(name="wpool", bufs=1) as wpool, \
             tc.tile_pool(name="xtp", bufs=6) as xtp, \
             tc.tile_pool(name="csp", bufs=6) as csp, \
             tc.tile_pool(name="qkp", bufs=8) as qkp, \
             tc.tile_pool(name="expp", bufs=8) as expp, \
             tc.tile_pool(name="outp", bufs=10) as outp, \
             tc.tile_pool(name="yop", bufs=2) as yop, \
             tc.tile_pool(name="ps_sc", bufs=2, space="PSUM") as ps_sc, \
             tc.tile_pool(name="ps_ot", bufs=2, space="PSUM") as ps_ot, \
             tc.tile_pool(name="ps_mm", bufs=2, space="PSUM") as ps_mm:

            # persistent tiles (weight DMAs are emitted after chunk 0's x
            # load so the first projection's data races ahead of them)
            ident = persist.tile([128, 128], F32, name="ident")
            make_identity(nc, ident)
            wq = wpool.tile([128, DC, CW], BF16, name="wq_sb")
            wk = wpool.tile([128, DC, CW], BF16, name="wk_sb")
            wv = wpool.tile([128, DC, CW], BF16, name="wv_sb")
            wo = wpool.tile([CW, DC, 128], BF16, name="wo_sb")
            bq = wpool.tile([CW, 1], F32, name="bq_sb")
            bk = wpool.tile([CW, 1], F32, name="bk_sb")

            def emit_weight_loads():
                # on the ACT hwdge queue: the sync queue is busy streaming x
                for wt, wdr in ((wk, wk_d), (wq, wq_d), (wv, wv_d)):
                    nc.scalar.dma_start(
                        out=wt, in_=wdr.ap().rearrange("p (kc m) -> p kc m", m=CW))
                nc.scalar.dma_start(
                    out=wo, in_=wo_d.ap().rearrange("p (mc m) -> p mc m", m=128))
                nc.scalar.dma_start(out=bq, in_=bq_d[:, :])
                nc.scalar.dma_start(out=bk, in_=bk_d[:, :])

            qtr = persist.tile([128, T], BF16, name="qtr")    # rope'd Q^T
            ktr = persist.tile([128, T], BF16, name="ktr")    # rope'd K^T
            aoT = persist.tile([128, T], BF16, name="aoT")    # normalized attn out^T
            # V natural per chunk: [128 tok, head, ktile-in-chunk, 64+1]
            vnat = [persist.tile([128, HPC, CH // 128, HD + 1], BF16, name=f"vnat{i}")
                    for i in range(NCHUNK)]
            # V staging (transpose input): per head, double-buffered by chunk
            # parity; row 64 is the ones row for the softmax denominator.
            vth = [[persist.tile([HD + 1, CH], F32, name=f"vth{h}{p}")
                    for p in range(2)] for h in range(HPC)]
            for h in range(HPC):
                for p in range(2):
                    nc.vector.memset(vth[h][p][HD:HD + 1, :], 1.0)

            # ---- chunk loads (immediate) + projection body (generator) ----
            loaded = {}

            def emit_chunk_load(n):
                # bf16 x: one 1MB DMA per chunk fits the sync queue's
                # bandwidth; cos/sin ride one packed DMA. Issues stay off
                # gpsimd (slow swdge + ucode-lib thrash vs partition
                # broadcasts) and mostly off ACT (the exp pacer).
                tcol = n * CH
                xta = xtp.tile([128, DC // 2, CH], BF16, name=f"xta{n}", tag="xt")
                xtb = xtp.tile([128, DC // 2, CH], BF16, name=f"xtb{n}", tag="xt")
                xview = xT_d.ap().rearrange("(dc p) t -> p dc t", p=128)
                nc.sync.dma_start(out=xta, in_=xview[:, 0:DC // 2, tcol:tcol + CH])
                nc.scalar.dma_start(out=xtb, in_=xview[:, DC // 2:DC, tcol:tcol + CH])
                cs = csp.tile([128, 2, CH], BF16, name=f"cs{n}", tag="cs")
                nc.sync.dma_start(
                    out=cs,
                    in_=cs_d.ap().rearrange("p (two t) -> p two t", two=2)[:, :, tcol:tcol + CH])
                loaded[n] = (xta, xtb, cs)

            def chunk_body(n, on_act, preload=None, defer_q=None):
                """Emit chunk n's QKV projections + rope, yielding between
                small bundles so attention slots interleave. on_act: put the
                PSUM->SBUF bias/copy work on ACT (else DVE). preload: issue
                chunk `preload`'s DMAs at the first bundle. defer_q: deque to
                push the q-part generator onto instead of emitting it inline
                (q of chunk 4+qc is only needed when group (1,qc) starts)."""
                if preload is not None:
                    emit_chunk_load(preload)
                tcol = n * CH
                xta, xtb, cs = loaded.pop(n)
                cos_c, sin_c = cs[:, 0, :], cs[:, 1, :]

                def xs(dc):
                    return (xta if dc < DC // 2 else xtb)[:, dc % (DC // 2), :]

                def qk_part(name, wt, bias, dst):
                    pp = ps_mm.tile([128, CH], F32, name=f"{name}pp{n}", tag="pp")
                    for dc in range(DC):
                        nc.tensor.matmul(pp[:, :], wt[:, dc, :], xs(dc),
                                         start=(dc == 0), stop=(dc == DC - 1))
                        if dc % 2 == 1:
                            yield
                    # bias + rope: dst = (pp+b)*cos + shuf(pp+b)*sin'
                    qs = qkp.tile([128, CH], BF16, name=f"{name}s{n}", tag="qs")
                    if on_act:
                        nc.scalar.activation(qs[:, :], pp[:, :], AF.Identity, bias=bias)
                    else:
                        nc.vector.tensor_scalar_add(qs[:, :], pp[:, :], bias[:, :])
                    qsw = qkp.tile([128, CH], BF16, name=f"{name}w{n}", tag="qs")
                    nc.vector.stream_shuffle(qsw[:, :], qs[:, :], SWAP_MASK)
                    t1 = qkp.tile([128, CH], BF16, name=f"{name}t1{n}", tag="qs")
                    t2 = qkp.tile([128, CH], BF16, name=f"{name}t2{n}", tag="qs")
                    nc.vector.tensor_mul(t1[:, :], qs[:, :], cos_c[:, :])
                    nc.vector.tensor_mul(t2[:, :], qsw[:, :], sin_c[:, :])
                    nc.vector.tensor_add(dst[:, tcol:tcol + CH], t1[:, :], t2[:, :])
                    yield

                def v_part():
                    pp = ps_mm.tile([128, CH], F32, name=f"vpp{n}", tag="pp")
                    for dc in range(DC):
                        nc.tensor.matmul(pp[:, :], wv[:, dc, :], xs(dc),
                                         start=(dc == 0), stop=(dc == DC - 1))
                        if dc % 2 == 1 and dc < DC - 1:
                            yield
                    for h in range(HPC):
                        if on_act:
                            nc.scalar.copy(vth[h][n % 2][0:HD, :], pp[HD * h:HD * (h + 1), :])
                        else:
                            nc.vector.tensor_copy(vth[h][n % 2][0:HD, :], pp[HD * h:HD * (h + 1), :])
                    yield
                    for h in range(HPC):
                        vp = ps_mm.tile([128, CH // 128, HD + 1], F32,
                                        name=f"vp{n}{h}", tag="pp")
                        for j in range(CH // 128):
                            nc.tensor.transpose(vp[:, j, :],
                                                vth[h][n % 2][:, 128 * j:128 * (j + 1)],
                                                ident[0:HD + 1, 0:HD + 1])
                        if on_act:
                            nc.scalar.copy(vnat[n][:, h, :, :], vp[:, :, :])
                        else:
                            nc.vector.tensor_copy(vnat[n][:, h, :, :], vp[:, :, :])
                        yield

                # K first (scores of this chunk's k-tiles unblock ASAP).
                # Chunk 0 needs Q immediately for the (b0,qc0) scores; for
                # later chunks Q is only used by future groups, so V (which
                # feeds the lag-delayed AVs) goes before Q.
                yield from qk_part("k", wk, bk, ktr)
                if n == 0:
                    yield from qk_part("q", wq, bq, qtr)
                    yield from v_part()
                elif defer_q is not None:
                    yield from v_part()
                    defer_q.append(qk_part("q", wq, bq, qtr))
                else:
                    yield from v_part()
                    yield from qk_part("q", wq, bq, qtr)

            # ---- attention slot: scores pair + exp for one k-tile ----
            def emit_scores(b, qc, kt):
                toff = b * S
                qcols = slice(toff + QW * qc, toff + QW * (qc + 1))
                krows = slice(toff + 128 * kt, toff + 128 * (kt + 1))
                sc = ps_sc.tile([128, 2 * QW], F32, name=f"sc{b}{qc}{kt}", tag="sc")
                for h in range(HPC):
                    po = HD * h
                    nc.tensor.matmul(sc[:, QW * h:QW * (h + 1)],
                                     ktr[po:po + HD, krows],
                                     qtr[po:po + HD, qcols], start=True, stop=True,
                                     tile_position=(po, 0))
                if use_mask:
                    mtile = expp.tile([128, QW], F32, name=f"mt{b}{qc}{kt}", tag="mt")
                    nc.sync.dma_start(
                        out=mtile,
                        in_=mt_d[128 * kt:128 * (kt + 1), QW * qc:QW * (qc + 1)])
                    for h in range(HPC):
                        nc.vector.tensor_scalar_mul(
                            sc[:, QW * h:QW * (h + 1)], sc[:, QW * h:QW * (h + 1)], SCALE)
                        nc.vector.tensor_add(
                            sc[:, QW * h:QW * (h + 1)], sc[:, QW * h:QW * (h + 1)],
                            mtile[:, :])
                ex = expp.tile([128, 2 * QW], BF16, name=f"ex{b}{qc}{kt}", tag="ex")
                nc.scalar.activation(ex[:, :], sc[:, :], AF.Exp,
                                     scale=(1.0 if use_mask else SCALE))
                return ex

            def emit_av(b, qc, kt, ex, ots):
                toff = b * S
                cn = (toff + 128 * kt) // CH
                j = (128 * kt % CH) // 128
                for h in range(HPC):
                    nc.tensor.matmul(ots[h][:, :], vnat[cn][:, h, j, :],
                                     ex[:, QW * h:QW * (h + 1)],
                                     start=(kt == 0), stop=(kt == KT - 1))

            def emit_norm(b, qc, ots):
                toff = b * S
                qcols = slice(toff + QW * qc, toff + QW * (qc + 1))
                # custom-DVE reciprocal needs SBUF input: copy den out of
                # PSUM first, broadcast, then invert (baseline-proven).
                # Stage-ordered across heads so gpsimd overlaps the DVE ops.
                dens, bcs, rcps = [], [], []
                for h in range(HPC):
                    den = outp.tile([1, QW], F32, name=f"den{b}{qc}{h}", tag="den")
                    nc.vector.tensor_copy(den[:, :], ots[h][HD:HD + 1, :])
                    dens.append(den)
                for h in range(HPC):
                    bc = outp.tile([HD, QW], F32, name=f"bc{b}{qc}{h}", tag="bc")
                    nc.gpsimd.partition_broadcast(bc[:, :], dens[h][:, :])
                    bcs.append(bc)
                for h in range(HPC):
                    rcp = outp.tile([HD, QW], F32, name=f"rcp{b}{qc}{h}", tag="rcp")
                    nc.vector.reciprocal_approx_fast(rcp[:, :], bcs[h][:, :])
                    rcps.append(rcp)
                for h in range(HPC):
                    po = HD * h
                    nc.vector.tensor_mul(aoT[po:po + HD, qcols],
                                         ots[h][0:HD, :], rcps[h][:, :])

            def oproj_gen(b, qc, act_half=False):
                toff = b * S
                qcols = slice(toff + QW * qc, toff + QW * (qc + 1))
                yog = yop.tile([128, DC, QW], BF16, name=f"yog{b}{qc}", tag="yog")
                yview = ypT_d.ap().rearrange("(mc p) t -> p mc t", p=128)
                for mo in range(DC):
                    yp = ps_mm.tile([128, QW], F32, name=f"yp{b}{qc}{mo}", tag="pp")
                    nc.tensor.matmul(yp[:, :], wo[:, mo, :], aoT[:, qcols],
                                     start=True, stop=True)
                    if act_half and mo % 2 == 1:
                        nc.scalar.copy(yog[:, mo, :], yp[:, :])
                    else:
                        nc.vector.tensor_copy(yog[:, mo, :], yp[:, :])
                    if mo == DC // 2 - 1:
                        nc.sync.dma_start(out=yview[:, 0:DC // 2, qcols],
                                          in_=yog[:, 0:DC // 2, :])
                    if mo % 2 == 1:
                        yield
                nc.sync.dma_start(out=yview[:, DC // 2:DC, qcols],
                                  in_=yog[:, DC // 2:DC, :])

            def pump1(q):
                while q:
                    try:
                        next(q[0])
                        return True
                    except StopIteration:
                        q.popleft()
                return False

            def drain(g):
                for _ in g:
                    pass

            # ---- emission schedule ----
            cfill = deque()    # chunk k/v projection generators
            qfill = deque()    # deferred q-projection generators (chunks 5-7)
            ofill = deque()    # o-proj generators

            def pump(k):
                for _ in range(k):
                    if not (pump1(cfill) or pump1(qfill) or pump1(ofill)):
                        return

            # Global slot stream: scores/exp lead, AV lags LAG slots behind
            # (across group boundaries too, so the exp stream never breaks
            # at a boundary); group norm + o-proj queue when its last AV
            # retires.
            LAG = 6
            pend = deque()

            def retire_one():
                if not pend:
                    return
                pb, pqc, pkt, pex, pots = pend.popleft()
                emit_av(pb, pqc, pkt, pex, pots)
                if pkt == KT - 1:
                    emit_norm(pb, pqc, pots)
                    last = (pb == 1 and pqc == QC - 1)
                    ofill.append(oproj_gen(pb, pqc, act_half=last))

            def gslot(b, qc, kt, ots):
                ex = emit_scores(b, qc, kt)
                if len(pend) >= LAG:
                    retire_one()
                pend.append((b, qc, kt, ex, ots))

            def new_ots(b, qc):
                return [ps_ot.tile([HD + 1, QW], F32, name=f"ot{b}{qc}{h}", tag="ot")
                        for h in range(HPC)]

            # Phase A: chunks 0-3 with (b0,qc0) attention interleaved;
            # x/cos chunk loads are prefetched one chunk ahead so the
            # projection matmuls never head-block the PE queue on a DMA.
            emit_chunk_load(0)
            emit_weight_loads()
            emit_chunk_load(1)
            # warm the PE pstate while chunk 0's x DMA is in flight: matmuls
            # on an on-chip constant tile ramp the clock from 0.65 to 2.4 GHz
            wtile = persist.tile([128, CH], BF16, name="warm")
            nc.vector.memset(wtile[:, :], 0.5)
            wps = ps_mm.tile([128, CH], F32, name="warmps", tag="pp")
            for _ in range(10):
                nc.tensor.matmul(wps[:, :], wtile[:, 0:128], wtile[:, :],
                                 start=True, stop=True)
            drain(chunk_body(0, on_act=True))
            ots0 = new_ots(0, 0)
            for n in range(1, 4):
                g = deque([chunk_body(n, on_act=False, preload=n + 1)])
                for _ in range(5):   # k-proj + k-rope of chunk n
                    pump1(g)
                for kt in range(4 * (n - 1), 4 * n):
                    gslot(0, 0, kt, ots0)
                    for _ in range(3):
                        pump1(g)
                while g:
                    pump1(g)
            # chunks 4-7 become fillers from here on (chunk 4 already loading)
            for n in range(4, NCHUNK):
                cfill.append(chunk_body(n, on_act=False,
                                        preload=(n + 1 if n + 1 < NCHUNK else None),
                                        defer_q=(qfill if n >= 5 else None)))
            for kt in range(12, KT):
                gslot(0, 0, kt, ots0)
                pump(2)

            # Phase B: (b0, qc1-3); fillers: chunks 4-7 (priority) + o-proj.
            for qc in range(1, QC):
                ots = new_ots(0, qc)
                for kt in range(KT):
                    gslot(0, qc, kt, ots)
                    pump(1)
            # Phase B2: (b1, qc0); chunk 7 must be fully emitted before its
            # k-tiles (12-15).
            ots = new_ots(1, 0)
            for kt in range(KT):
                if kt == 12:
                    while cfill:
                        pump1(cfill)
                gslot(1, 0, kt, ots)
                pump(2 if kt < 12 else 1)
            # Phase C: (b1, qc1-3); fillers: deferred q-parts + o-proj.
            for qc in range(1, QC):
                # group (1,qc) needs qtr of chunk 4+qc: finish its q-part
                while len(qfill) > QC - 1 - qc:
                    if not pump1(qfill):
                        break
                ots = new_ots(1, qc)
                last = (qc == QC - 1)
                for kt in range(KT):
                    gslot(1, qc, kt, ots)
                    if last and kt >= KT - LAG:
                        retire_one()   # drain the AV backlog before the end
                        retire_one()
                    pump(1)
            while pend:
                retire_one()
            while ofill:
                pump1(ofill)

    nc.compile()
    nc.m = get_hw_module(nc.m)
    return nc


def _get_nc(use_mask: bool):
    key = ("nc", use_mask)
    if key not in _CACHE:
        _CACHE[key] = _build(use_mask)
    return _CACHE[key]


def kernel(x, rope, mask, Wq, bq, Wk, bk, Wv, bv, Wo, bo, _trace=False):
    import ml_dtypes
    x = np.asarray(x, dtype=np.float32)
    rope = np.asarray(rope, dtype=np.float32)
    mask = np.asarray(mask, dtype=np.float32)
    Wq = np.asarray(Wq, dtype=np.float32)
    Wk = np.asarray(Wk, dtype=np.float32)
    Wv = np.asarray(Wv, dtype=np.float32)
    Wo = np.asarray(Wo, dtype=np.float32)
    use_mask = bool(np.any(mask))

    xT = np.ascontiguousarray(x.reshape(T, D).T).astype(ml_dtypes.bfloat16)
    cos = rope[0, 0, :, 0, :]                                     # [S, 64]
    sin = rope[1, 0, :, 0, :]
    sgn = np.where(np.arange(HD) % 2 == 0, -1.0, 1.0).astype(np.float32)[:, None]
    cosT = np.tile(cos.T, (HPC, B))
    sinT = np.tile(sin.T * sgn, (HPC, B))
    csT = np.ascontiguousarray(
        np.concatenate([cosT, sinT], axis=1)).astype(ml_dtypes.bfloat16)

    nc = _get_nc(use_mask)

    def warr(W, cs):
        # [D, CW] -> [128, DC*CW]: partition p holds rows p, 128+p, ... so
        # the device DMA is a contiguous-row load
        return np.ascontiguousarray(
            W[:, cs].reshape(DC, 128, CW).transpose(1, 0, 2)
            .reshape(128, DC * CW)).astype(ml_dtypes.bfloat16)

    in_maps = []
    for c in range(NC):
        cs = slice(CW * c, CW * (c + 1))
        m = dict(
            xt=xT,
            wq=warr(Wq, cs),
            bq=np.ascontiguousarray(bq[cs]).reshape(CW, 1).astype(np.float32),
            wk=warr(Wk, cs),
            bk=np.ascontiguousarray(bk[cs]).reshape(CW, 1).astype(np.float32),
            wv=warr(Wv, cs),
            wo=np.ascontiguousarray(Wo[cs, :]).astype(ml_dtypes.bfloat16),
            cst=csT,
        )
        if use_mask:
            m["maskt"] = np.ascontiguousarray(mask[0, 0].T).astype(np.float32)
        in_maps.append(m)

    # transient device wedges (NRT_EXEC_UNIT_UNRECOVERABLE) clear on retry
    last_err = None
    for _attempt in range(3):
        try:
            res = bass_utils.run_bass_kernel_spmd(
                nc, in_maps, core_ids=list(range(NC)), trace=_trace)
            break
        except Exception as e:  # noqa: BLE001
            last_err = e
            import time as _time
            _time.sleep(2.0)
    else:
        raise last_err
    # row-parallel unshard: sum the per-core bf16 partials in fp32, add the
    # output bias and the folded V bias (bv commutes through attention).
    ypT = res.results[0]["ypT"].astype(np.float32)
    for c in range(1, NC):
        ypT = ypT + res.results[c]["ypT"].astype(np.float32)
    bo_eff = np.asarray(bo, dtype=np.float32) + \
        np.asarray(bv, dtype=np.float32) @ Wo
    out = (ypT.T + bo_eff).reshape(B, S, D).astype(np.float32)
    out = np.ascontiguousarray(out)
    if _trace:
        return out, res
    return out

